# revision 1
# baseline (speedup 1.0000x reference)
"""Trainium2 Bass kernel for Convpass-swintransformer hypernet-mask adapter.

Data-parallel over batch: 8 NeuronCores x 8 samples each. All weights
replicated; x is host-transposed to channel-major so every on-device matmul
consumes natural layouts (no on-device transposes).
"""
import sys

sys.path.insert(0, "/opt/trn_rl_repo")

import numpy as np

import concourse.bass as bass
import concourse.tile as tile
from concourse import bacc, mybir
from concourse.bass_utils import run_bass_kernel_spmd

AF = mybir.ActivationFunctionType
FP32 = mybir.dt.float32

B, L, C = 64, 784, 384
DIM, NM, META = 64, 16, 64
HH, WW = 28, 28
NCORES = 8
S = B // NCORES          # samples per core
KC = C // 128            # 3 contraction chunks for C=384
NPOS = [(0, 512), (512, 272)]   # 784 split at psum-bank boundary
QSCALE = 1.702

_CACHE = {}


def _build_nc():
    nc = bacc.Bacc(None)
    d = nc.declare_dram_parameter
    xt_d = d("xt", [S, KC, 128, L], FP32, isOutput=False)
    wa_d = d("wa", [KC, 128, 128], FP32, isOutput=False)
    ba_d = d("ba", [1, 128], FP32, isOutput=False)
    mw2_d = d("mw2", [META, META], FP32, isOutput=False)
    mb2_d = d("mb2", [1, META], FP32, isOutput=False)
    mtT_d = d("mtT", [META, NM], FP32, isOutput=False)
    upw_d = d("upw", [DIM, C], FP32, isOutput=False)
    upb_d = d("upb", [1, C], FP32, isOutput=False)
    hw_d = d("hw", [128, 9, 2048], FP32, isOutput=False)
    hb_d = d("hb", [64, 576], FP32, isOutput=False)
    out_d = d("out", [S, L, C], FP32, isOutput=True)

    with tile.TileContext(nc) as tc:
        with tc.tile_pool(name="consts", bufs=1) as cp, \
             tc.tile_pool(name="xdp", bufs=S) as xdp, \
             tc.tile_pool(name="cwall", bufs=1) as cwp, \
             tc.tile_pool(name="cws", bufs=S) as cwsp:
            # ---- constants ----
            wa = cp.tile([128, KC, 128], FP32)
            nc.sync.dma_start(out=wa[:], in_=wa_d[:].rearrange("k p m -> p k m"))
            ba = cp.tile([1, 128], FP32)
            nc.sync.dma_start(out=ba[:], in_=ba_d[:])
            mw2 = cp.tile([META, META], FP32)
            nc.sync.dma_start(out=mw2[:], in_=mw2_d[:])
            mb2 = cp.tile([1, META], FP32)
            nc.sync.dma_start(out=mb2[:], in_=mb2_d[:])
            mtT = cp.tile([META, NM], FP32)
            nc.sync.dma_start(out=mtT[:], in_=mtT_d[:])
            upw = cp.tile([DIM, C], FP32)
            nc.sync.dma_start(out=upw[:], in_=upw_d[:])
            upb = cp.tile([1, C], FP32)
            nc.sync.dma_start(out=upb[:], in_=upb_d[:])
            hb = cp.tile([64, 576], FP32)
            nc.sync.dma_start(out=hb[:], in_=hb_d[:])
            ones1 = cp.tile([1, L], FP32)
            nc.vector.memset(ones1[:], 1.0)
            ones16 = cp.tile([NM, 64], FP32)
            nc.vector.memset(ones16[:], 1.0)
            featT2 = cp.tile([128, 32], FP32)
            nc.vector.memset(featT2[:], 0.0)

            xd_tiles = []

            # ================= phase A: meta-net / masks / feat =============
            with tc.tile_pool(name="xtp", bufs=3) as xtp, \
                 tc.tile_pool(name="psA", bufs=2, space="PSUM") as psA, \
                 tc.tile_pool(name="psB", bufs=2, space="PSUM") as psB, \
                 tc.tile_pool(name="sbA", bufs=3) as sbA, \
                 tc.tile_pool(name="smallA", bufs=2) as smA:
                for s in range(S):
                    xt = xtp.tile([128, KC, L], FP32, tag="xt")
                    nc.sync.dma_start(
                        out=xt[:], in_=xt_d[s].rearrange("k p q -> p k q"))
                    psa = psA.tile([128, L], FP32, tag="psa")
                    for n0, nw in NPOS:
                        for k in range(KC):
                            nc.tensor.matmul(
                                psa[:, n0:n0 + nw], lhsT=wa[:, k, :],
                                rhs=xt[:, k, n0:n0 + nw],
                                start=(k == 0), stop=False)
                        nc.tensor.matmul(
                            psa[:, n0:n0 + nw], lhsT=ba[:],
                            rhs=ones1[:, n0:n0 + nw], start=False, stop=True)
                    h = sbA.tile([META, L], FP32, tag="h")
                    nc.scalar.activation(h[:], psa[0:META, :], AF.Relu)
                    xd = xdp.tile([DIM, L], FP32)
                    nc.vector.tensor_copy(xd[:], psa[META:128, :])
                    xd_tiles.append(xd)

                    psp = psB.tile([META, L], FP32, tag="psb")
                    for n0, nw in NPOS:
                        nc.tensor.matmul(psp[:, n0:n0 + nw], lhsT=mw2[:],
                                         rhs=h[:, n0:n0 + nw],
                                         start=True, stop=False)
                        nc.tensor.matmul(psp[:, n0:n0 + nw], lhsT=mb2[:],
                                         rhs=ones1[:, n0:n0 + nw],
                                         start=False, stop=True)
                    prompt = sbA.tile([META, L], FP32, tag="prompt")
                    nc.scalar.activation(prompt[:], psp[:], AF.Copy)

                    psm = psB.tile([NM, L], FP32, tag="psb")
                    for n0, nw in NPOS:
                        nc.tensor.matmul(psm[:, n0:n0 + nw], lhsT=mtT[:],
                                         rhs=prompt[:, n0:n0 + nw],
                                         start=True, stop=True)
                    expt = sbA.tile([NM, L], FP32, tag="expt")
                    zsum = smA.tile([NM, 1], FP32, tag="z")
                    nc.scalar.activation(expt[:], psm[:], AF.Exp,
                                         accum_out=zsum[:])
                    invz = smA.tile([NM, 1], FP32, tag="iz")
                    nc.vector.reciprocal(invz[:], zsum[:])
                    expn = sbA.tile([NM, L], FP32, tag="expn")
                    nc.vector.tensor_scalar_mul(expn[:], expt[:], invz[:])

                    pss = psB.tile([64, L], FP32, tag="psb")
                    for n0, nw in NPOS:
                        nc.tensor.matmul(pss[:, n0:n0 + nw], lhsT=ones16[:],
                                         rhs=expn[:, n0:n0 + nw],
                                         start=True, stop=True)
                    ftmp = sbA.tile([64, L], FP32, tag="ftmp")
                    nc.vector.tensor_mul(ftmp[:], pss[:], prompt[:])
                    nc.vector.reduce_sum(featT2[0:64, s:s + 1], ftmp[:],
                                         axis=mybir.AxisListType.X)

            # duplicate feats into upper partition half for split-K hypernet
            nc.sync.dma_start(out=featT2[64:128, :], in_=featT2[0:64, :])

            # ================= phase H: hypernet conv weights ===============
            # Column block j8 of a sample's conv weight lands in psum group
            # g=j8//3 at base partition 32*(j8%3)+s (psum matmul base must be
            # in {0,32,64}).  Whole-tile drains; junk partitions never read.
            GROUPS = [(0, 3), (3, 3), (6, 2)]   # (first j8, blocks) per group
            cwalls = [cwp.tile([32 * nb, 9 * 512], FP32,
                               name=f"cwall{g}", tag=f"cwall{g}")
                      for g, (_, nb) in enumerate(GROUPS)]
            with tc.tile_pool(name="hwp", bufs=2) as hwp, \
                 tc.tile_pool(name="psH", bufs=6, space="PSUM") as psH:
                for n9 in range(9):
                    hwc = hwp.tile([128, 2048], FP32, tag="hw")
                    nc.scalar.dma_start(out=hwc[:], in_=hw_d[:, n9, :])
                    for g, (j8_0, nb) in enumerate(GROUPS):
                        psh = psH.tile([32 * nb, 512], FP32, tag="psh")
                        for slot in range(nb):
                            j8 = j8_0 + slot
                            h2, j4 = divmod(j8, 4)
                            nc.tensor.matmul(
                                psh[32 * slot:32 * slot + 32, :],
                                lhsT=featT2[h2 * 64:(h2 + 1) * 64, :],
                                rhs=hwc[h2 * 64:(h2 + 1) * 64,
                                        j4 * 512:(j4 + 1) * 512],
                                start=True, stop=True)
                        if n9 % 2 == 0:
                            nc.vector.tensor_copy(
                                cwalls[g][:, n9 * 512:(n9 + 1) * 512], psh[:])
                        else:
                            nc.scalar.activation(
                                cwalls[g][:, n9 * 512:(n9 + 1) * 512], psh[:],
                                AF.Copy)

            cw_tiles = []
            for s in range(S):
                cw = cwsp.tile([64, 576], FP32)
                for g, (j8_0, nb) in enumerate(GROUPS):
                    for slot in range(nb):
                        j8 = j8_0 + slot
                        p0 = 32 * slot + s
                        nc.sync.dma_start(
                            out=cw[8 * j8:8 * (j8 + 1), :],
                            in_=cwalls[g][p0:p0 + 1].rearrange(
                                "p (a b) -> p a b", a=8))
                nc.vector.tensor_add(cw[:], cw[:], hb[:])
                cw_tiles.append(cw)

            # ================= phase B: adapter conv + up ===================
            with tc.tile_pool(name="padp", bufs=2) as padp, \
                 tc.tile_pool(name="sgp", bufs=2) as sgp, \
                 tc.tile_pool(name="yap", bufs=2) as yap, \
                 tc.tile_pool(name="outp", bufs=3) as outp, \
                 tc.tile_pool(name="psC0", bufs=2, space="PSUM") as psC0, \
                 tc.tile_pool(name="psC1", bufs=2, space="PSUM") as psC1, \
                 tc.tile_pool(name="psU", bufs=4, space="PSUM") as psU:
                for s in range(S):
                    xd = xd_tiles[s]
                    pad = padp.tile([64, 900], FP32, tag="pad")
                    nc.gpsimd.memset(pad[:], 0.0)
                    sg = sgp.tile([DIM, L], FP32, tag="sg")
                    nc.scalar.activation(sg[:], xd[:], AF.Sigmoid, scale=QSCALE)
                    pad3 = pad.rearrange("p (r c) -> p r c", r=30)
                    nc.vector.tensor_mul(
                        pad3[:, 1:29, 1:29],
                        sg.rearrange("p (a b) -> p a b", a=28)[:],
                        xd.rearrange("p (a b) -> p a b", a=28)[:])

                    ps0 = psC0.tile([64, 448], FP32, tag="c0")
                    ps1 = psC1.tile([64, 336], FP32, tag="c1")
                    cwv = cw_tiles[s].rearrange("p (o k) -> p k o", k=9)
                    for k9 in range(9):
                        ky, kx = divmod(k9, 3)
                        lw = cwv[:, k9, :]
                        nc.tensor.matmul(
                            ps0[:], lhsT=lw,
                            rhs=pad3[:, ky:ky + 16, kx:kx + 28],
                            start=(k9 == 0), stop=(k9 == 8))
                        nc.tensor.matmul(
                            ps1[:], lhsT=lw,
                            rhs=pad3[:, ky + 16:ky + 28, kx:kx + 28],
                            start=(k9 == 0), stop=(k9 == 8))

                    ya = yap.tile([DIM, L], FP32, tag="ya")
                    ys0 = sgp.tile([64, 448], FP32, tag="ys0")
                    nc.scalar.activation(ys0[:], ps0[:], AF.Sigmoid, scale=QSCALE)
                    nc.vector.tensor_mul(ya[:, 0:448], ys0[:], ps0[:])
                    ys1 = sgp.tile([64, 336], FP32, tag="ys1")
                    nc.scalar.activation(ys1[:], ps1[:], AF.Sigmoid, scale=QSCALE)
                    nc.vector.tensor_mul(ya[:, 448:784], ys1[:], ps1[:])

                    outt = outp.tile([112, 7, C], FP32, tag="outt")
                    for j in range(7):
                        psu = psU.tile([112, C], FP32, tag="psu")
                        nc.tensor.matmul(psu[:], lhsT=ya[:, j * 112:(j + 1) * 112],
                                         rhs=upw[:], start=True, stop=False)
                        nc.tensor.matmul(psu[:], lhsT=ones1[:, 0:112],
                                         rhs=upb[:], start=False, stop=True)
                        if j % 2 == 0:
                            nc.scalar.activation(outt[:, j, :], psu[:], AF.Copy)
                        else:
                            nc.vector.tensor_copy(outt[:, j, :], psu[:])
                    nc.sync.dma_start(
                        out=out_d[s].rearrange("(j p) c -> p j c", p=112),
                        in_=outt[:])
    nc.finalize()
    return nc


def _prep(x, meta_w1, meta_b1, meta_w2, meta_b2, mask_token,
          hyper_w, hyper_b, down_w, down_b, up_w, up_b):
    f = lambda a: np.ascontiguousarray(np.asarray(a, dtype=np.float32))
    x = f(x)
    xt = np.ascontiguousarray(x.reshape(B, L, C).transpose(0, 2, 1))  # [B,C,L]
    xt = xt.reshape(B, KC, 128, L)

    wA = np.concatenate([f(meta_w1), f(down_w)], axis=1)        # [384,128]
    wa = np.ascontiguousarray(wA.reshape(KC, 128, 128))
    ba = np.concatenate([f(meta_b1), f(down_b)])[None, :]       # [1,128]
    mtT = np.ascontiguousarray(f(mask_token).T)                 # [64,16]

    hw5 = f(hyper_w).reshape(META, DIM, DIM, 3, 3)
    hwr = np.ascontiguousarray(hw5.transpose(0, 2, 1, 3, 4)).reshape(
        META, 8, 8, 576).reshape(META, 8, 9, 512)               # [n,j8,n9,c]
    top = hwr[:, 0:4].transpose(0, 2, 1, 3)                     # [64,9,4,512]
    bot = hwr[:, 4:8].transpose(0, 2, 1, 3)
    hw128 = np.ascontiguousarray(
        np.concatenate([top, bot], axis=0)).reshape(128, 9, 2048)

    hb5 = f(hyper_b).reshape(DIM, DIM, 3, 3)
    hbcw = np.ascontiguousarray(hb5.transpose(1, 0, 2, 3)).reshape(64, 576)

    consts = {
        "wa": wa, "ba": np.ascontiguousarray(ba),
        "mw2": f(meta_w2), "mb2": f(meta_b2)[None, :],
        "mtT": mtT, "upw": f(up_w), "upb": f(up_b)[None, :],
        "hw": hw128, "hb": hbcw,
    }
    in_maps = []
    for c in range(NCORES):
        m = dict(consts)
        m["xt"] = np.ascontiguousarray(xt[c * S:(c + 1) * S])
        in_maps.append(m)
    return in_maps


def _run(in_maps, **kw):
    if "nc" not in _CACHE:
        _CACHE["nc"] = _build_nc()
    return run_bass_kernel_spmd(_CACHE["nc"], in_maps, list(range(NCORES)), **kw)


def kernel(x, meta_w1, meta_b1, meta_w2, meta_b2, mask_token,
           hyper_w, hyper_b, down_w, down_b, up_w, up_b, H, W):
    assert int(H) == HH and int(W) == WW
    in_maps = _prep(x, meta_w1, meta_b1, meta_w2, meta_b2, mask_token,
                    hyper_w, hyper_b, down_w, down_b, up_w, up_b)
    res = _run(in_maps)
    out = np.concatenate([res.results[c]["out"] for c in range(NCORES)], axis=0)
    return out.reshape(B, L, C).astype(np.float32)

